# revision 1
# baseline (speedup 1.0000x reference)
"""DKEPooling Trainium2 kernel.

Per-graph SNR-scaled gaussian perturbation + covariance + Newton-Schulz
matrix sqrt + cov^(1/2) @ mean, data-parallel over 8 NeuronCores
(16 graphs per core; B=128, n=128 nodes/graph, d=256 features).

Math restructuring (verified exact vs reference in fp32):
  P     = feat + s * noise                  (s = sqrt(tvar/nvar/10^1.5))
  diff  = P - colmean(P)                    (graph-mean of noise cancels)
  A     = diff^T diff / ||diff||_F^2        (the 1/(n-1) cancels in A)
  tr    = ||diff||_F^2 / (n-1)
  NS iterations with U = 1.5 I - 0.5 T fused into the PSUM->SBUF copy
  out   = YZY @ ((colmean(P) - s*mean(noise)) * sqrt(tr))

All NS matrices are symmetric (polynomials in A) so lhsT = the matrix
itself; no transposes needed.  Large matmuls run as float32r (full-rate
fp32 PE path for N>=256; plain fp32 is 4 cycles/row) — fp32r operands
must be produced as fp32r, so matmul-feeding tiles are declared f32r.
Reduction/accumulation tiles stay fp32 and their (tiny) matmuls run fp32.
"""

import os
import sys
from contextlib import ExitStack

sys.path.insert(0, "/opt/trn_rl_repo")

import numpy as np

import concourse.bass as bass
import concourse.bacc as bacc
import concourse.tile as tile
from concourse import mybir
from concourse.bass_utils import run_bass_kernel_spmd

N_CORES = 8
B, NNODE, D = 128, 128, 256
GPC = B // N_CORES            # graphs per core
NTOT = float(NNODE * D)       # entries per graph
SNR_FACTOR = 10.0 ** (-15.0 / 10.0)  # 10^(-SNR/10)

F32 = mybir.dt.float32
F32R = mybir.dt.float32r
TS = bass.ts
AX = mybir.AxisListType
OP = mybir.AluOpType
AF = mybir.ActivationFunctionType

# Module-level stash for test.py introspection (exec time / profile).
LAST_RESULTS = None


def _inline_tensor_typed(nc, data, name, dtype):
    """nc.inline_tensor with an explicit mybir dtype (e.g. float32r)."""
    import base64
    import io

    data = np.ascontiguousarray(data)
    mls = nc._tensor(name, list(data.shape), dtype, kind="Const", type="DRAM")
    buf = io.BytesIO()
    np.save(buf, data, allow_pickle=False)
    mls.file = f"{name}.npy"
    mls.ant_data = base64.standard_b64encode(buf.getvalue()).decode()
    return bass.DRamTensorHandle(name, list(data.shape), dtype)


def _build_bass():
    nc = bacc.Bacc("TRN2", target_bir_lowering=False, debug=False)
    feat_d = nc.dram_tensor("feat", [GPC * NNODE, D], F32R, kind="ExternalInput")
    noise_d = nc.dram_tensor("noise", [GPC * NNODE, D], F32R, kind="ExternalInput")
    out_d = nc.dram_tensor("out", [GPC, D], F32, kind="ExternalOutput")

    i15_np = np.zeros((128, 2, 256), np.float32)
    for m in range(2):
        for p in range(128):
            i15_np[p, m, m * 128 + p] = 1.5
    i15_d = nc.inline_tensor(i15_np, "i15const")
    oonsq_np = np.full((128, 128), 1.0 / 128.0, np.float32)
    oonsq_d = _inline_tensor_typed(nc, oonsq_np, "oonsqconst", F32R)
    ident_np = np.eye(128, dtype=np.float32)
    ident_d = _inline_tensor_typed(nc, ident_np, "identconst", F32R)
    d3_np = np.zeros((128, 2, 256), np.float32)
    for m in range(2):
        for p in range(128):
            d3_np[p, m, m * 128 + p] = -3.0
    d3_d = _inline_tensor_typed(nc, d3_np, "d3const", F32R)

    reps = int(os.environ.get("DKE_REPS", "1"))
    unroll = os.environ.get("DKE_REPS_MODE", "loop") == "unroll"
    with tile.TileContext(nc) as tc:
        if reps > 1 and not unroll:
            with tc.For_i(0, reps, 1):
                _build_tile(nc, tc, feat_d, noise_d, i15_d, oonsq_d, ident_d, d3_d, out_d)
        else:
            for _ in range(reps):
                _build_tile(nc, tc, feat_d, noise_d, i15_d, oonsq_d, ident_d, d3_d, out_d)
    nc.compile()
    return nc


def _build_tile(nc, tc, feat_d, noise_d, i15_d, oonsq_d, ident_d, d3_d, out_d):
    fv = feat_d[:, :].rearrange("(g n) d -> g n d", n=NNODE)
    nv = noise_d[:, :].rearrange("(g n) d -> g n d", n=NNODE)
    NGRP = 2
    GSZ = GPC // NGRP

    with ExitStack() as ctx:
        consts = ctx.enter_context(tc.tile_pool(name="consts", bufs=1))
        tpool = ctx.enter_context(tc.tile_pool(name="tpool", bufs=GPC))
        stats = ctx.enter_context(tc.tile_pool(name="stats", bufs=1))
        scratch = ctx.enter_context(tc.tile_pool(name="scratch", bufs=3))
        work = ctx.enter_context(tc.tile_pool(name="work", bufs=6))
        nsp = ctx.enter_context(tc.tile_pool(name="nsp", bufs=6))
        small = ctx.enter_context(tc.tile_pool(name="small", bufs=18))
        # two PSUM pools selected by graph parity -> adjacent graphs never
        # contend for banks, enabling 2-way cross-graph overlap
        psA = ctx.enter_context(tc.tile_pool(name="psA", bufs=4, space="PSUM"))
        psB = ctx.enter_context(tc.tile_pool(name="psB", bufs=4, space="PSUM"))

        # ---- constants ----
        oon128f = consts.tile([128, 1], F32, tag="oon128f")
        nc.vector.memset(oon128f, 1.0 / NNODE)
        oon_sq = consts.tile([128, 128], F32R, tag="oon_sq")
        nc.sync.dma_start(out=oon_sq, in_=oonsq_d[:, :])
        ones128f = consts.tile([128, 1], F32, tag="ones128f")
        nc.vector.memset(ones128f, 1.0)
        ones1f = consts.tile([1, 128], F32, tag="ones1f")
        nc.vector.memset(ones1f, 1.0)
        onesSQ = consts.tile([128, 128], F32, tag="onesSQ")
        nc.vector.memset(onesSQ, 1.0)
        i15 = consts.tile([128, 2, 256], F32, tag="i15")
        nc.sync.dma_start(out=i15, in_=i15_d[:, :, :])
        ident128 = consts.tile([128, 128], F32R, tag="ident128")
        nc.sync.dma_start(out=ident128, in_=ident_d[:, :])
        d3 = consts.tile([128, 2, 256], F32R, tag="d3")
        nc.sync.dma_start(out=d3, in_=d3_d[:, :, :])

        # ---- persistent stat rows (per stats group) ----
        rows = [stats.tile([128, GSZ, 2], F32, tag="rows", name=f"rows{k}")
                for k in range(NGRP)]
        rsums = [stats.tile([1, GSZ, 2], F32, tag="rsums", name=f"rsums{k}")
                 for k in range(NGRP)]
        srow = [stats.tile([1, GSZ, 2], F32, tag="srow", name=f"srow{k}")
                for k in range(NGRP)]
        sc_all = [stats.tile([128, GSZ, 2], F32, tag="sc_all", name=f"sc{k}")
                  for k in range(NGRP)]
        out_all = stats.tile([128, GPC * 2], F32, tag="out_all")

        def load_and_accum(g):
            grp, j = divmod(g, GSZ)
            T = tpool.tile([128, 2, 256], F32R, tag="T", name="T")
            nc.sync.dma_start(out=T[:, 0, :], in_=fv[g])
            nc.sync.dma_start(out=T[:, 1, :], in_=nv[g])
            Ftile, Nztile = T[:, 0, :], T[:, 1, :]
            pool = psA if g % 2 == 0 else psB
            cs = pool.tile([1, 512], F32, tag="ps", name="cs")
            nc.tensor.matmul(cs, ones128f.bitcast(F32R), T[:, :, :],
                             start=True, stop=True)
            nc.vector.tensor_reduce(
                out=rsums[grp][0:1, j, :],
                in_=cs.rearrange("a (b c) -> a b c", c=256),
                axis=AX.X, op=OP.add)
            scr = scratch.tile([128, 256], F32, tag="sqscrA", name="scrA")
            nc.scalar.activation(out=scr, in_=Ftile, func=AF.Square,
                                 accum_out=rows[grp][:, j, 0:1])
            scr = scratch.tile([128, 256], F32, tag="sqscrA", name="scrA")
            nc.scalar.activation(out=scr, in_=Nztile, func=AF.Square,
                                 accum_out=rows[grp][:, j, 1:2])
            return T

        def stats_group(grp, pool):
            # partition-sum all rows for the group, then batched scalar math
            tot_ps = pool.tile([1, GSZ * 2], F32, tag="ps", name="tot_ps")
            nc.tensor.matmul(tot_ps, ones128f, rows[grp][:, :, :],
                             start=True, stop=True)
            rview = rsums[grp].rearrange("a g t -> a (g t)")
            sums_sq = small.tile([1, 2 * GSZ], F32, tag="ssq", name="ssq")
            nc.vector.tensor_mul(sums_sq, rview, rview)
            varnum = small.tile([1, 2 * GSZ], F32, tag="vn", name="vn")
            nc.vector.scalar_tensor_tensor(
                out=varnum, in0=sums_sq, scalar=-1.0 / NTOT,
                in1=tot_ps, op0=OP.mult, op1=OP.add)
            vview = varnum.rearrange("a (g t) -> a g t", t=2)
            rnv = small.tile([1, GSZ, 1], F32, tag="rnv", name="rnv")
            nc.vector.reciprocal(rnv, vview[:, :, 1:2])
            ratio = small.tile([1, GSZ, 1], F32, tag="ratio", name="ratio")
            nc.vector.tensor_mul(ratio, vview[:, :, 0:1], rnv)
            nc.scalar.activation(
                out=srow[grp][:, :, 0:1], in_=ratio, func=AF.Sqrt,
                scale=SNR_FACTOR)
            t3 = small.tile([1, GSZ, 1], F32, tag="t3", name="t3")
            nc.vector.tensor_mul(t3, srow[grp][:, :, 0:1], rsums[grp][:, :, 1:2])
            nc.scalar.mul(out=srow[grp][:, :, 1:2], in_=t3, mul=1.0 / NTOT)
            sc_ps = pool.tile([128, GSZ * 2], F32, tag="ps", name="sc_ps")
            nc.tensor.matmul(sc_ps, ones1f, srow[grp][0:1, :, :],
                             start=True, stop=True)
            nc.scalar.copy(out=sc_all[grp],
                           in_=sc_ps.rearrange("p (g t) -> p g t", t=2))

        # =============== Phase A: load + stats (grouped) ===============
        Ts = []
        for grp in range(NGRP):
            for j in range(GSZ):
                Ts.append(load_and_accum(grp * GSZ + j))
            stats_group(grp, psA if grp % 2 == 0 else psB)

        # =============== Phase B: two passes ===============
        # Pass 1 emits every graph's pre-NS work (independent across graphs),
        # pass 2 emits the 16 independent NS chains; this gives the static
        # scheduler a deep pool of ready work on every engine.
        def t_mms(pool, dsts, YZt):
            # T = Z@Y - 3I : the block-identity matmul accumulates -3I so
            # the PSUM drain (U = -0.5*T) is a pure scale on either engine.
            for m in range(2):
                for kc in range(2):
                    nc.tensor.matmul(
                        dsts[m], YZt[kc][:, 256 + 128 * m : 384 + 128 * m],
                        YZt[kc][:, 0:256],
                        start=(kc == 0), stop=False)
                nc.tensor.matmul(
                    dsts[m], ident128, d3[:, m, :], start=False, stop=True)

        def w_mms(dsts, U, YZt):
            # W = U@[Y|Z] : lhsT = U chunks, rhs = full [128,512]
            for m in range(2):
                for kc in range(2):
                    nc.tensor.matmul(
                        dsts[m], U[kc][:, TS(m, 128)], YZt[kc],
                        start=(kc == 0), stop=(kc == 1))

        def ps_pair(pool, width=256):
            return [pool.tile([128, width], F32, tag="ps", name="nsps")
                    for _ in range(2)]

        copy_ctr = [0]

        def cp_alt(dst, src):
            copy_ctr[0] += 1
            if copy_ctr[0] % 2:
                nc.scalar.copy(out=dst, in_=src)
            else:
                nc.vector.tensor_copy(out=dst, in_=src)

        state = []
        for g in range(GPC):
            grp, j = divmod(g, GSZ)
            pool = psA if g % 2 == 0 else psB
            T = Ts[g]
            Ftile, Nztile = T[:, 0, :], T[:, 1, :]
            s128 = sc_all[grp][:, j, 0:1]
            sgm128 = sc_all[grp][:, j, 1:2]

            # P = F + s * Nz
            P = work.tile([128, 256], F32R, tag="P", name="P")
            nc.vector.scalar_tensor_tensor(
                out=P, in0=Nztile, scalar=s128, in1=Ftile,
                op0=OP.mult, op1=OP.add)

            # column-mean of P broadcast via (1/n) ones matrix; diff = P - mean
            bcast = pool.tile([128, 256], F32, tag="ps", name="bcast")
            nc.tensor.matmul(bcast, oon_sq, P, start=True, stop=True)
            diff = work.tile([128, 256], F32R, tag="diff", name="diff")
            nc.vector.tensor_sub(diff, P, bcast)

            # mean' column (fp32 matvec)
            mean_ps = pool.tile([128, 2], F32, tag="ps", name="mean_ps")
            for m in range(2):
                nc.tensor.matmul(
                    mean_ps[:, m : m + 1], P.bitcast(F32)[:, TS(m, 128)],
                    oon128f, start=True, stop=True)

            # trace rows = sum(diff^2); broadcast tr to all partitions (PE)
            scr = scratch.tile([128, 256], F32, tag="sqscrA", name="scrA")
            trrows = small.tile([128, 1], F32, tag="trrows", name="trrows")
            nc.scalar.activation(out=scr, in_=diff, func=AF.Square,
                                 accum_out=trrows)
            trps = pool.tile([128, 1], F32, tag="ps", name="trps")
            nc.tensor.matmul(trps, onesSQ, trrows, start=True, stop=True)
            rtr128 = small.tile([128, 1], F32, tag="rtr", name="rtr")
            nc.vector.reciprocal(rtr128, trps)
            sqtr128 = small.tile([128, 1], F32, tag="sqtr", name="sqtr")
            nc.scalar.activation(
                out=sqtr128, in_=trps, func=AF.Sqrt, scale=1.0 / (NNODE - 1))

            # mv = (mean' - sgm) * sqtr
            mv = small.tile([128, 2], F32, tag="mv", name="mv")
            nc.vector.tensor_scalar(
                out=mv, in0=mean_ps, scalar1=sgm128, scalar2=sqtr128,
                op0=OP.subtract, op1=OP.mult)

            # cov = diff^T diff ; A = cov * rtr
            covp = pool.tile([128, 512], F32, tag="ps", name="covp")
            for m in range(2):
                nc.tensor.matmul(covp[:, TS(m, 256)], diff[:, TS(m, 128)], diff,
                                 start=True, stop=True)
            Afull = nsp.tile([128, 512], F32R, tag="nsa", name="Afull")
            nc.scalar.mul(out=Afull, in_=covp, mul=rtr128)
            A = [Afull[:, TS(m, 256)] for m in range(2)]

            # it0: U0 = 1.5I - 0.5A -> Z half of YZ ; Y1 = U0@A -> Y half
            YZ = [nsp.tile([128, 512], F32R, tag="ns", name="YZ", bufs=40)
                  for _ in range(2)]
            for m in range(2):
                nc.vector.scalar_tensor_tensor(
                    out=YZ[m][:, 256:512], in0=A[m], scalar=-0.5,
                    in1=i15[:, m, :], op0=OP.mult, op1=OP.add)
            Yp = ps_pair(pool)
            for m in range(2):
                for kc in range(2):
                    nc.tensor.matmul(
                        Yp[m], YZ[kc][:, 256 + 128 * m : 384 + 128 * m], A[kc],
                        start=(kc == 0), stop=(kc == 1))
            cp_alt(YZ[0][:, 0:256], Yp[0])
            cp_alt(YZ[1][:, 0:256], Yp[1])
            state.append((pool, YZ, mv))

        YZs = [st[1] for st in state]
        for it in range(3):
            for g in range(GPC):
                pool = state[g][0]
                Tp = ps_pair(pool)
                t_mms(pool, Tp, YZs[g])
                U = [nsp.tile([128, 256], F32R, tag="nsu", name="U")
                     for _ in range(2)]
                nc.vector.tensor_scalar_mul(out=U[0], in0=Tp[0], scalar1=-0.5)
                nc.scalar.mul(out=U[1], in_=Tp[1], mul=-0.5)
                Wp = ps_pair(pool, 512)
                w_mms(Wp, U, YZs[g])
                YZn = [nsp.tile([128, 512], F32R, tag="ns", name="YZ", bufs=40)
                       for _ in range(2)]
                cp_alt(YZn[0], Wp[0])
                cp_alt(YZn[1], Wp[1])
                YZs[g] = YZn

        for g in range(GPC):
            pool, _, mv = state[g]
            YZ = YZs[g]
            Tp = ps_pair(pool)
            t_mms(pool, Tp, YZ)
            U = [nsp.tile([128, 256], F32R, tag="nsu", name="U")
                 for _ in range(2)]
            nc.vector.tensor_scalar_mul(out=U[0], in0=Tp[0], scalar1=-0.5)
            nc.scalar.mul(out=U[1], in_=Tp[1], mul=-0.5)
            Yp = ps_pair(pool)
            for m in range(2):
                for kc in range(2):
                    nc.tensor.matmul(
                        Yp[m], U[kc][:, TS(m, 128)], YZ[kc][:, 0:256],
                        start=(kc == 0), stop=(kc == 1))
            YZY = [nsp.tile([128, 256], F32R, tag="nsu", name="YZY")
                   for _ in range(2)]
            cp_alt(YZY[0], Yp[0])
            cp_alt(YZY[1], Yp[1])

            outp = pool.tile([128, 2], F32, tag="ps", name="outp")
            for m in range(2):
                for kc in range(2):
                    nc.tensor.matmul(
                        outp[:, m : m + 1],
                        YZY[kc].bitcast(F32)[:, TS(m, 128)],
                        mv[:, kc : kc + 1],
                        start=(kc == 0), stop=(kc == 1))
            nc.vector.tensor_copy(out=out_all[:, 2 * g : 2 * g + 2], in_=outp)

        # single output DMA: out[g, m*128+p] <- out_all[p, 2g+m]
        nc.sync.dma_start(
            out=out_d[:, :].rearrange("g (m p) -> p g m", p=128),
            in_=out_all.rearrange("p (g m) -> p g m", m=2),
        )


_NC_CACHE = None


def kernel(**inputs):
    global _NC_CACHE, LAST_RESULTS
    feat = np.ascontiguousarray(inputs["feat"], dtype=np.float32)
    noise = np.ascontiguousarray(inputs["noise"], dtype=np.float32)
    assert feat.shape == (B * NNODE, D) and noise.shape == (B * NNODE, D)

    if _NC_CACHE is None:
        _NC_CACHE = _build_bass()
    nc = _NC_CACHE

    rows = GPC * NNODE
    in_maps = [
        {
            "feat": feat[c * rows : (c + 1) * rows],
            "noise": noise[c * rows : (c + 1) * rows],
        }
        for c in range(N_CORES)
    ]
    res = run_bass_kernel_spmd(
        nc,
        in_maps,
        core_ids=list(range(N_CORES)),
        trace=bool(int(os.environ.get("DKE_TRACE", "0"))),
    )
    LAST_RESULTS = res
    out = np.concatenate([m["out"] for m in res.results], axis=0)
    return out.astype(np.float32)


if __name__ == "__main__":
    rng = np.random.default_rng(0)
    ins = {
        "batch_list": np.full((B,), NNODE, np.int32),
        "feat": rng.standard_normal((B * NNODE, D)).astype(np.float32),
        "noise": rng.standard_normal((B * NNODE, D)).astype(np.float32),
    }
    o = kernel(**ins)
    print(o.shape, o.dtype, np.abs(o).max())



# revision 2
# speedup vs baseline: 1.0958x; 1.0958x over previous
"""DKEPooling Trainium2 kernel.

Per-graph SNR-scaled gaussian perturbation + covariance + Newton-Schulz
matrix sqrt + cov^(1/2) @ mean, data-parallel over 8 NeuronCores
(16 graphs per core; B=128, n=128 nodes/graph, d=256 features).

Math restructuring (verified exact vs reference in fp32):
  P     = feat + s * noise                  (s = sqrt(tvar/nvar/10^1.5))
  diff  = P - colmean(P)                    (graph-mean of noise cancels)
  A     = diff^T diff / ||diff||_F^2        (the 1/(n-1) cancels in A)
  tr    = ||diff||_F^2 / (n-1)
  NS iterations with U = 1.5 I - 0.5 T fused into the PSUM->SBUF copy
  out   = YZY @ ((colmean(P) - s*mean(noise)) * sqrt(tr))

All NS matrices are symmetric (polynomials in A) so lhsT = the matrix
itself; no transposes needed.  Large matmuls run as float32r (full-rate
fp32 PE path for N>=256; plain fp32 is 4 cycles/row) — fp32r operands
must be produced as fp32r, so matmul-feeding tiles are declared f32r.
Reduction/accumulation tiles stay fp32 and their (tiny) matmuls run fp32.
"""

import os
import sys
from contextlib import ExitStack

sys.path.insert(0, "/opt/trn_rl_repo")

import numpy as np

import concourse.bass as bass
import concourse.bacc as bacc
import concourse.tile as tile
from concourse import mybir
from concourse.bass_utils import run_bass_kernel_spmd

N_CORES = 8
B, NNODE, D = 128, 128, 256
GPC = B // N_CORES            # graphs per core
NTOT = float(NNODE * D)       # entries per graph
SNR_FACTOR = 10.0 ** (-15.0 / 10.0)  # 10^(-SNR/10)

F32 = mybir.dt.float32
F32R = mybir.dt.float32r
TS = bass.ts
AX = mybir.AxisListType
OP = mybir.AluOpType
AF = mybir.ActivationFunctionType

# Module-level stash for test.py introspection (exec time / profile).
LAST_RESULTS = None


def _inline_tensor_typed(nc, data, name, dtype):
    """nc.inline_tensor with an explicit mybir dtype (e.g. float32r)."""
    import base64
    import io

    data = np.ascontiguousarray(data)
    mls = nc._tensor(name, list(data.shape), dtype, kind="Const", type="DRAM")
    buf = io.BytesIO()
    np.save(buf, data, allow_pickle=False)
    mls.file = f"{name}.npy"
    mls.ant_data = base64.standard_b64encode(buf.getvalue()).decode()
    return bass.DRamTensorHandle(name, list(data.shape), dtype)


def _build_bass():
    nc = bacc.Bacc("TRN2", target_bir_lowering=False, debug=False)
    feat_d = nc.dram_tensor("feat", [GPC * NNODE, D], F32R, kind="ExternalInput")
    noise_d = nc.dram_tensor("noise", [GPC * NNODE, D], F32R, kind="ExternalInput")
    out_d = nc.dram_tensor("out", [128, GPC * 2], F32, kind="ExternalOutput")

    i15_np = np.zeros((128, 2, 256), np.float32)
    for m in range(2):
        for p in range(128):
            i15_np[p, m, m * 128 + p] = 1.5
    i15_d = nc.inline_tensor(i15_np, "i15const")
    oonsq_np = np.full((128, 128), 1.0 / 128.0, np.float32)
    oonsq_d = _inline_tensor_typed(nc, oonsq_np, "oonsqconst", F32R)
    ident_np = np.eye(128, dtype=np.float32)
    ident_d = _inline_tensor_typed(nc, ident_np, "identconst", F32R)
    d3_np = np.zeros((128, 2, 256), np.float32)
    for m in range(2):
        for p in range(128):
            d3_np[p, m, m * 128 + p] = -3.0
    d3_d = _inline_tensor_typed(nc, d3_np, "d3const", F32R)

    reps = int(os.environ.get("DKE_REPS", "1"))
    unroll = os.environ.get("DKE_REPS_MODE", "loop") == "unroll"
    with tile.TileContext(nc) as tc:
        if reps > 1 and not unroll:
            with tc.For_i(0, reps, 1):
                _build_tile(nc, tc, feat_d, noise_d, i15_d, oonsq_d, ident_d, d3_d, out_d)
        else:
            for _ in range(reps):
                _build_tile(nc, tc, feat_d, noise_d, i15_d, oonsq_d, ident_d, d3_d, out_d)
    nc.compile()
    return nc


def _build_tile(nc, tc, feat_d, noise_d, i15_d, oonsq_d, ident_d, d3_d, out_d):
    fv = feat_d[:, :].rearrange("(g n) d -> g n d", n=NNODE)
    nv = noise_d[:, :].rearrange("(g n) d -> g n d", n=NNODE)
    NGRP = 2
    GSZ = GPC // NGRP

    with ExitStack() as ctx:
        consts = ctx.enter_context(tc.tile_pool(name="consts", bufs=1))
        tpool = ctx.enter_context(tc.tile_pool(name="tpool", bufs=GPC))
        stats = ctx.enter_context(tc.tile_pool(name="stats", bufs=1))
        scratch = ctx.enter_context(tc.tile_pool(name="scratch", bufs=3))
        work = ctx.enter_context(tc.tile_pool(name="work", bufs=6))
        nsp = ctx.enter_context(tc.tile_pool(name="nsp", bufs=6))
        small = ctx.enter_context(tc.tile_pool(name="small", bufs=24))
        # one global 8-slot PSUM ring (8 banks x 2KB); slots recycle FIFO
        ps8 = ctx.enter_context(tc.tile_pool(name="ps8", bufs=8, space="PSUM"))

        # ---- constants ----
        oon128f = consts.tile([128, 1], F32, tag="oon128f")
        nc.vector.memset(oon128f, 1.0 / NNODE)
        oon_sq = consts.tile([128, 128], F32R, tag="oon_sq")
        nc.sync.dma_start(out=oon_sq, in_=oonsq_d[:, :])
        ones128f = consts.tile([128, 1], F32, tag="ones128f")
        nc.vector.memset(ones128f, 1.0)
        ones1f = consts.tile([1, 128], F32, tag="ones1f")
        nc.vector.memset(ones1f, 1.0)
        onesSQ = consts.tile([128, 128], F32, tag="onesSQ")
        nc.vector.memset(onesSQ, 1.0)
        i15 = consts.tile([128, 2, 256], F32, tag="i15")
        nc.sync.dma_start(out=i15, in_=i15_d[:, :, :])
        ident128 = consts.tile([128, 128], F32R, tag="ident128")
        nc.sync.dma_start(out=ident128, in_=ident_d[:, :])
        d3c = consts.tile([128, 2, 256], F32R, tag="d3c")
        nc.sync.dma_start(out=d3c, in_=d3_d[:, :, :])

        # ---- persistent stat rows (per stats group) ----
        rows = [stats.tile([128, GSZ, 2], F32, tag="rows", name=f"rows{k}")
                for k in range(NGRP)]
        rsums = [stats.tile([1, GSZ, 2], F32, tag="rsums", name=f"rsums{k}")
                 for k in range(NGRP)]
        srow = [stats.tile([1, GSZ, 2], F32, tag="srow", name=f"srow{k}")
                for k in range(NGRP)]
        sc_all = [stats.tile([128, GSZ, 2], F32, tag="sc_all", name=f"sc{k}")
                  for k in range(NGRP)]
        out_all = stats.tile([128, GPC * 2], F32, tag="out_all")

        def load_and_accum(g):
            grp, j = divmod(g, GSZ)
            T = tpool.tile([128, 2, 256], F32R, tag="T", name="T")
            nc.sync.dma_start(out=T[:, 0, :], in_=fv[g])
            nc.sync.dma_start(out=T[:, 1, :], in_=nv[g])
            Ftile, Nztile = T[:, 0, :], T[:, 1, :]
            cs = ps8.tile([1, 512], F32, tag="ps", name="cs")
            nc.tensor.matmul(cs, ones128f.bitcast(F32R), T[:, :, :],
                             start=True, stop=True)
            nc.vector.tensor_reduce(
                out=rsums[grp][0:1, j, :],
                in_=cs.rearrange("a (b c) -> a b c", c=256),
                axis=AX.X, op=OP.add)
            scr = scratch.tile([128, 256], F32, tag="sqscrA", name="scrA")
            nc.scalar.activation(out=scr, in_=Ftile, func=AF.Square,
                                 accum_out=rows[grp][:, j, 0:1])
            scr = scratch.tile([128, 256], F32, tag="sqscrA", name="scrA")
            nc.scalar.activation(out=scr, in_=Nztile, func=AF.Square,
                                 accum_out=rows[grp][:, j, 1:2])
            return T

        def stats_group(grp):
            # partition-sum all rows for the group, then batched scalar math
            tot_ps = ps8.tile([1, GSZ * 2], F32, tag="ps", name="tot_ps")
            nc.tensor.matmul(tot_ps, ones128f, rows[grp][:, :, :],
                             start=True, stop=True)
            rview = rsums[grp].rearrange("a g t -> a (g t)")
            sums_sq = small.tile([1, 2 * GSZ], F32, tag="ssq", name="ssq")
            nc.vector.tensor_mul(sums_sq, rview, rview)
            varnum = small.tile([1, 2 * GSZ], F32, tag="vn", name="vn")
            nc.vector.scalar_tensor_tensor(
                out=varnum, in0=sums_sq, scalar=-1.0 / NTOT,
                in1=tot_ps, op0=OP.mult, op1=OP.add)
            vview = varnum.rearrange("a (g t) -> a g t", t=2)
            rnv = small.tile([1, GSZ, 1], F32, tag="rnv", name="rnv")
            nc.vector.reciprocal(rnv, vview[:, :, 1:2])
            ratio = small.tile([1, GSZ, 1], F32, tag="ratio", name="ratio")
            nc.vector.tensor_mul(ratio, vview[:, :, 0:1], rnv)
            nc.scalar.activation(
                out=srow[grp][:, :, 0:1], in_=ratio, func=AF.Sqrt,
                scale=SNR_FACTOR)
            t3 = small.tile([1, GSZ, 1], F32, tag="t3", name="t3")
            nc.vector.tensor_mul(t3, srow[grp][:, :, 0:1], rsums[grp][:, :, 1:2])
            nc.scalar.mul(out=srow[grp][:, :, 1:2], in_=t3, mul=1.0 / NTOT)
            sc_ps = ps8.tile([128, GSZ * 2], F32, tag="ps", name="sc_ps")
            nc.tensor.matmul(sc_ps, ones1f, srow[grp][0:1, :, :],
                             start=True, stop=True)
            nc.scalar.copy(out=sc_all[grp],
                           in_=sc_ps.rearrange("p (g t) -> p g t", t=2))

        # =============== Phase A: load + stats (grouped) ===============
        Ts = []
        for grp in range(NGRP):
            for j in range(GSZ):
                Ts.append(load_and_accum(grp * GSZ + j))
            stats_group(grp)

        # =============== Phase B: two passes ===============
        # Pass 1 emits every graph's pre-NS work (independent across graphs),
        # pass 2 emits the 16 independent NS chains; this gives the static
        # scheduler a deep pool of ready work on every engine.
        def t_mms(Tp, YZt):
            # T = Z@Y into one [128,512] psum tile (halves m=0/m=1).
            # m=1 half: accumulate -3I on the PE so its drain is a pure
            # -0.5 scale on either engine; m=0 drains as STT on DVE.
            for kc in range(2):
                nc.tensor.matmul(
                    Tp[:, 0:256], YZt[kc][:, 256:384],
                    YZt[kc][:, 0:256],
                    start=(kc == 0), stop=(kc == 1))
            for kc in range(2):
                nc.tensor.matmul(
                    Tp[:, 256:512], YZt[kc][:, 384:512],
                    YZt[kc][:, 0:256],
                    start=(kc == 0), stop=False)
            nc.tensor.matmul(Tp[:, 256:512], ident128, d3c[:, 1, :],
                             start=False, stop=True)

        udrain_ctr = [0]

        def u_drain(U, Tp):
            # U = 1.5I - 0.5*T ; m=0 via DVE STT, m=1 (PE pre-subtracted 3I)
            # is a pure scale -- alternate it between Act and DVE.
            nc.vector.scalar_tensor_tensor(
                out=U[0], in0=Tp[:, 0:256], scalar=-0.5,
                in1=i15[:, 0, :], op0=OP.mult, op1=OP.add)
            udrain_ctr[0] += 1
            if udrain_ctr[0] % 2:
                nc.scalar.mul(out=U[1], in_=Tp[:, 256:512], mul=-0.5)
            else:
                nc.vector.tensor_scalar_mul(out=U[1], in0=Tp[:, 256:512],
                                            scalar1=-0.5)

        def w_mms(dsts, U, YZt):
            # W = U@[Y|Z] : lhsT = U chunks, rhs = full [128,512]
            for m in range(2):
                for kc in range(2):
                    nc.tensor.matmul(
                        dsts[m], U[kc][:, TS(m, 128)], YZt[kc],
                        start=(kc == 0), stop=(kc == 1))

        copy_ctr = [0]

        def cp_alt(dst, src):
            copy_ctr[0] += 1
            if copy_ctr[0] % 2:
                nc.scalar.copy(out=dst, in_=src)
            else:
                nc.vector.tensor_copy(out=dst, in_=src)

        # ---- pass 1, stage-major over blocks of 4 graphs: each engine's
        # queue gets 4 independent ops per stage, hiding dep latency ----
        state = [None] * GPC
        p1 = {}
        BLK = 4

        def p1_stageP(g):
            grp, j = divmod(g, GSZ)
            T = Ts[g]
            P = work.tile([128, 256], F32R, tag="P", name="P", bufs=8)
            nc.vector.scalar_tensor_tensor(
                out=P, in0=T[:, 1, :], scalar=sc_all[grp][:, j, 0:1],
                in1=T[:, 0, :], op0=OP.mult, op1=OP.add)
            p1[g] = {"P": P}

        def p1_bcast(g):
            bcast = ps8.tile([128, 256], F32, tag="ps", name="bcast")
            nc.tensor.matmul(bcast, oon_sq, p1[g]["P"], start=True, stop=True)
            p1[g]["bcast"] = bcast

        def p1_diff(g):
            P = p1[g]["P"]
            diff = work.tile([128, 256], F32R, tag="diff", name="diff", bufs=8)
            nc.vector.tensor_sub(diff, P, p1[g]["bcast"])
            p1[g]["diff"] = diff

        def p1_meansq(g):
            # mean (cols 0:2) and trace partial (col 2) pack into one psum
            P = p1[g]["P"]
            mtp = ps8.tile([128, 4], F32, tag="ps", name="mtp")
            for m in range(2):
                nc.tensor.matmul(
                    mtp[:, m : m + 1], P.bitcast(F32)[:, TS(m, 128)],
                    oon128f, start=True, stop=True)
            p1[g]["mtp"] = mtp
            scr = scratch.tile([128, 256], F32, tag="sqscrA", name="scrA")
            trrows = small.tile([128, 1], F32, tag="trrows", name="trrows")
            nc.scalar.activation(out=scr, in_=p1[g]["diff"], func=AF.Square,
                                 accum_out=trrows)
            p1[g]["trrows"] = trrows

        def p1_trace(g):
            mtp = p1[g]["mtp"]
            nc.tensor.matmul(mtp[:, 2:3], onesSQ, p1[g]["trrows"],
                             start=True, stop=True)
            rtr128 = small.tile([128, 1], F32, tag="rtr", name="rtr")
            nc.vector.reciprocal(rtr128, mtp[:, 2:3])
            sqtr128 = small.tile([128, 1], F32, tag="sqtr", name="sqtr")
            nc.scalar.activation(
                out=sqtr128, in_=mtp[:, 2:3], func=AF.Sqrt,
                scale=1.0 / (NNODE - 1))
            p1[g]["rtr"] = rtr128
            p1[g]["sqtr"] = sqtr128

        def p1_mv_cov(g):
            grp, j = divmod(g, GSZ)
            mv = small.tile([128, 2], F32, tag="mv", name="mv")
            nc.vector.tensor_scalar(
                out=mv, in0=p1[g]["mtp"][:, 0:2],
                scalar1=sc_all[grp][:, j, 1:2],
                scalar2=p1[g]["sqtr"], op0=OP.subtract, op1=OP.mult)
            p1[g]["mv"] = mv
            diff = p1[g]["diff"]
            covp = ps8.tile([128, 512], F32, tag="ps", name="covp")
            for m in range(2):
                nc.tensor.matmul(covp[:, TS(m, 256)], diff[:, TS(m, 128)],
                                 diff, start=True, stop=True)
            p1[g]["covp"] = covp

        def p1_a_u0(g):
            Afull = nsp.tile([128, 512], F32R, tag="nsa", name="Afull", bufs=8)
            nc.scalar.mul(out=Afull, in_=p1[g]["covp"], mul=p1[g]["rtr"])
            A = [Afull[:, TS(m, 256)] for m in range(2)]
            YZ = [nsp.tile([128, 512], F32R, tag="ns", name="YZ", bufs=40)
                  for _ in range(2)]
            for m in range(2):
                nc.vector.scalar_tensor_tensor(
                    out=YZ[m][:, 256:512], in0=A[m], scalar=-0.5,
                    in1=i15[:, m, :], op0=OP.mult, op1=OP.add)
            p1[g]["A"] = A
            p1[g]["YZ"] = YZ

        def p1_y1(g):
            A, YZ = p1[g]["A"], p1[g]["YZ"]
            Yp = ps8.tile([128, 512], F32, tag="ps", name="Yp")
            for m in range(2):
                for kc in range(2):
                    nc.tensor.matmul(
                        Yp[:, TS(m, 256)],
                        YZ[kc][:, 256 + 128 * m : 384 + 128 * m], A[kc],
                        start=(kc == 0), stop=(kc == 1))
            cp_alt(YZ[0][:, 0:256], Yp[:, 0:256])
            cp_alt(YZ[1][:, 0:256], Yp[:, 256:512])
            state[g] = (YZ, p1[g]["mv"])

        P1_STAGES = [p1_stageP, p1_bcast, p1_diff, p1_meansq, p1_trace,
                     p1_mv_cov, p1_a_u0, p1_y1]

        YZs = [None] * GPC
        U3s = [None] * GPC

        def unit_pass1(b):
            for stage in P1_STAGES:
                for g in range(b * BLK, (b + 1) * BLK):
                    stage(g)
            for g in range(b * BLK, (b + 1) * BLK):
                YZs[g] = state[g][0]

        # full coupled iteration: T = Z@Y ; U = 1.5I-0.5T ; [Y|Z] = U@[Y|Z]
        def unit_iter(b):
            gs = range(b * BLK, (b + 1) * BLK)
            Tps, Us = {}, {}
            for g in gs:
                Tp = ps8.tile([128, 512], F32, tag="ps", name="Tp")
                t_mms(Tp, YZs[g])
                Tps[g] = Tp
            for g in gs:
                U = [nsp.tile([128, 256], F32R, tag="nsu", name="U", bufs=40)
                     for _ in range(2)]
                u_drain(U, Tps[g])
                Us[g] = U
            Wps = {}
            for g in gs:
                Wp = [ps8.tile([128, 512], F32, tag="ps", name="Wp")
                      for _ in range(2)]
                w_mms(Wp, Us[g], YZs[g])
                Wps[g] = Wp
            for g in gs:
                YZn = [nsp.tile([128, 512], F32R, tag="ns", name="YZ",
                                bufs=40) for _ in range(2)]
                cp_alt(YZn[0], Wps[g][0])
                cp_alt(YZn[1], Wps[g][1])
                YZs[g] = YZn

        # final stage (v2-style matrix form, stage-major within block):
        # T = Z4@Y4 ; U = 1.5I-0.5T ; YZY = U@Y4 ; out = YZY@mv
        def unit_final(b):
            gs = range(b * BLK, (b + 1) * BLK)
            Tps, Us, Yps, YZYs = {}, {}, {}, {}
            for g in gs:
                Tp = ps8.tile([128, 512], F32, tag="ps", name="Tp")
                t_mms(Tp, YZs[g])
                Tps[g] = Tp
            for g in gs:
                U = [nsp.tile([128, 256], F32R, tag="nsu", name="U", bufs=40)
                     for _ in range(2)]
                u_drain(U, Tps[g])
                Us[g] = U
            for g in gs:
                Yp = ps8.tile([128, 512], F32, tag="ps", name="Yp")
                for m in range(2):
                    for kc in range(2):
                        nc.tensor.matmul(
                            Yp[:, TS(m, 256)], Us[g][kc][:, TS(m, 128)],
                            YZs[g][kc][:, 0:256],
                            start=(kc == 0), stop=(kc == 1))
                Yps[g] = Yp
            for g in gs:
                YZY = [nsp.tile([128, 256], F32R, tag="nsu", name="YZY",
                                bufs=40) for _ in range(2)]
                cp_alt(YZY[0], Yps[g][:, 0:256])
                cp_alt(YZY[1], Yps[g][:, 256:512])
                YZYs[g] = YZY
            for g in gs:
                mv = state[g][1]
                outp = ps8.tile([128, 2], F32, tag="ps", name="outp")
                for m in range(2):
                    for kc in range(2):
                        nc.tensor.matmul(
                            outp[:, m : m + 1],
                            YZYs[g][kc].bitcast(F32)[:, TS(m, 128)],
                            mv[:, kc : kc + 1],
                            start=(kc == 0), stop=(kc == 1))
                cp_alt(out_all[:, 2 * g : 2 * g + 2], outp)

        # ---- software-pipelined wavefront: interleave the elementwise-
        # heavy pass1 with the PE-heavy NS iterations across blocks ----
        NB = GPC // BLK
        UNITS = [unit_pass1, unit_iter, unit_iter, unit_iter, unit_final]
        K = len(UNITS)
        for t in range(NB + K - 1):
            for k in range(K - 1, -1, -1):
                b = t - k
                if 0 <= b < NB:
                    UNITS[k](b)

        # contiguous output DMA; host reorders to [GPC, D]
        nc.sync.dma_start(out=out_d[:, :], in_=out_all)


_NC_CACHE = None


def kernel(**inputs):
    global _NC_CACHE, LAST_RESULTS
    feat = np.ascontiguousarray(inputs["feat"], dtype=np.float32)
    noise = np.ascontiguousarray(inputs["noise"], dtype=np.float32)
    assert feat.shape == (B * NNODE, D) and noise.shape == (B * NNODE, D)

    if _NC_CACHE is None:
        _NC_CACHE = _build_bass()
    nc = _NC_CACHE

    rows = GPC * NNODE
    in_maps = [
        {
            "feat": feat[c * rows : (c + 1) * rows],
            "noise": noise[c * rows : (c + 1) * rows],
        }
        for c in range(N_CORES)
    ]
    res = run_bass_kernel_spmd(
        nc,
        in_maps,
        core_ids=list(range(N_CORES)),
        trace=bool(int(os.environ.get("DKE_TRACE", "0"))),
    )
    LAST_RESULTS = res
    # buf[p, 2g+m] -> out[g, m*128+p]
    out = np.concatenate(
        [
            m["out"].reshape(128, GPC, 2).transpose(1, 2, 0).reshape(GPC, D)
            for m in res.results
        ],
        axis=0,
    )
    return out.astype(np.float32)


if __name__ == "__main__":
    rng = np.random.default_rng(0)
    ins = {
        "batch_list": np.full((B,), NNODE, np.int32),
        "feat": rng.standard_normal((B * NNODE, D)).astype(np.float32),
        "noise": rng.standard_normal((B * NNODE, D)).astype(np.float32),
    }
    o = kernel(**ins)
    print(o.shape, o.dtype, np.abs(o).max())



# revision 4
# speedup vs baseline: 1.3039x; 1.1899x over previous
"""DKEPooling Trainium2 kernel.

Per-graph SNR-scaled gaussian perturbation + covariance + Newton-Schulz
matrix sqrt + cov^(1/2) @ mean, data-parallel over 8 NeuronCores
(16 graphs per core; B=128, n=128 nodes/graph, d=256 features).

Math restructuring (verified exact vs reference in fp32):
  P     = feat + s * noise                  (s = sqrt(tvar/nvar/10^1.5))
  diff  = P - colmean(P)                    (graph-mean of noise cancels)
  A     = diff^T diff / ||diff||_F^2        (the 1/(n-1) cancels in A)
  tr    = ||diff||_F^2 / (n-1)
  NS iterations with U = 1.5 I - 0.5 T fused into the PSUM->SBUF copy
  out   = YZY @ ((colmean(P) - s*mean(noise)) * sqrt(tr))

All NS matrices are symmetric (polynomials in A) so lhsT = the matrix
itself; no transposes needed.  Large matmuls run as float32r (full-rate
fp32 PE path for N>=256; plain fp32 is 4 cycles/row) — fp32r operands
must be produced as fp32r, so matmul-feeding tiles are declared f32r.
Reduction/accumulation tiles stay fp32 and their (tiny) matmuls run fp32.
"""

import os
import sys
from contextlib import ExitStack

sys.path.insert(0, "/opt/trn_rl_repo")

import numpy as np

import concourse.bass as bass
import concourse.bacc as bacc
import concourse.tile as tile
from concourse import mybir
from concourse.bass_utils import run_bass_kernel_spmd

N_CORES = 8
B, NNODE, D = 128, 128, 256
GPC = B // N_CORES            # graphs per core
NTOT = float(NNODE * D)       # entries per graph
SNR_FACTOR = 10.0 ** (-15.0 / 10.0)  # 10^(-SNR/10)

F32 = mybir.dt.float32
F32R = mybir.dt.float32r
BF16 = mybir.dt.bfloat16
TS = bass.ts
AX = mybir.AxisListType
OP = mybir.AluOpType
AF = mybir.ActivationFunctionType

# Module-level stash for test.py introspection (exec time / profile).
LAST_RESULTS = None


def _inline_tensor_typed(nc, data, name, dtype):
    """nc.inline_tensor with an explicit mybir dtype (e.g. float32r)."""
    import base64
    import io

    data = np.ascontiguousarray(data)
    mls = nc._tensor(name, list(data.shape), dtype, kind="Const", type="DRAM")
    buf = io.BytesIO()
    np.save(buf, data, allow_pickle=False)
    mls.file = f"{name}.npy"
    mls.ant_data = base64.standard_b64encode(buf.getvalue()).decode()
    return bass.DRamTensorHandle(name, list(data.shape), dtype)


def _build_bass():
    nc = bacc.Bacc("TRN2", target_bir_lowering=False, debug=False)
    feat_d = nc.dram_tensor("feat", [GPC * NNODE, D], BF16, kind="ExternalInput")
    noise_d = nc.dram_tensor("noise", [GPC * NNODE, D], BF16, kind="ExternalInput")
    out_d = nc.dram_tensor("out", [128, GPC * 2], F32, kind="ExternalOutput")

    i15_np = np.zeros((128, 2, 256), np.float32)
    for m in range(2):
        for p in range(128):
            i15_np[p, m, m * 128 + p] = 1.5
    i15_d = nc.inline_tensor(i15_np, "i15const")
    oonsq_np = np.full((128, 128), 1.0 / 128.0, np.float32)
    oonsq_d = _inline_tensor_typed(nc, oonsq_np, "oonsqconst", F32R)
    ident_np = np.eye(128, dtype=np.float32)
    ident_d = _inline_tensor_typed(nc, ident_np, "identconst", F32R)
    d3_np = np.zeros((128, 2, 256), np.float32)
    for m in range(2):
        for p in range(128):
            d3_np[p, m, m * 128 + p] = -3.0
    d3_d = _inline_tensor_typed(nc, d3_np, "d3const", F32R)

    reps = int(os.environ.get("DKE_REPS", "1"))
    unroll = os.environ.get("DKE_REPS_MODE", "loop") == "unroll"
    with tile.TileContext(nc) as tc:
        if reps > 1 and not unroll:
            with tc.For_i(0, reps, 1):
                _build_tile(nc, tc, feat_d, noise_d, i15_d, oonsq_d, ident_d, d3_d, out_d)
        else:
            for _ in range(reps):
                _build_tile(nc, tc, feat_d, noise_d, i15_d, oonsq_d, ident_d, d3_d, out_d)
    nc.compile()
    return nc


def _build_tile(nc, tc, feat_d, noise_d, i15_d, oonsq_d, ident_d, d3_d, out_d):
    fv = feat_d[:, :].rearrange("(g n) d -> g n d", n=NNODE)
    nv = noise_d[:, :].rearrange("(g n) d -> g n d", n=NNODE)
    NGRP = 2
    GSZ = GPC // NGRP

    with ExitStack() as ctx:
        consts = ctx.enter_context(tc.tile_pool(name="consts", bufs=1))
        tpool = ctx.enter_context(tc.tile_pool(name="tpool", bufs=GPC))
        stats = ctx.enter_context(tc.tile_pool(name="stats", bufs=1))
        scratch = ctx.enter_context(tc.tile_pool(name="scratch", bufs=3))
        work = ctx.enter_context(tc.tile_pool(name="work", bufs=6))
        nsp = ctx.enter_context(tc.tile_pool(name="nsp", bufs=6))
        small = ctx.enter_context(tc.tile_pool(name="small", bufs=24))
        # one global 8-slot PSUM ring (8 banks x 2KB); slots recycle FIFO
        ps8 = ctx.enter_context(tc.tile_pool(name="ps8", bufs=8, space="PSUM"))

        # ---- constants ----
        oon128f = consts.tile([128, 1], F32, tag="oon128f")
        nc.vector.memset(oon128f, 1.0 / NNODE)
        oon_sq = consts.tile([128, 128], F32R, tag="oon_sq")
        nc.sync.dma_start(out=oon_sq, in_=oonsq_d[:, :])
        ones128f = consts.tile([128, 1], F32, tag="ones128f")
        nc.vector.memset(ones128f, 1.0)
        ones128b = consts.tile([128, 1], BF16, tag="ones128b")
        nc.vector.memset(ones128b, 1.0)
        ones1f = consts.tile([1, 128], F32, tag="ones1f")
        nc.vector.memset(ones1f, 1.0)
        onesSQ = consts.tile([128, 128], F32, tag="onesSQ")
        nc.vector.memset(onesSQ, 1.0)
        i15 = consts.tile([128, 2, 256], F32, tag="i15")
        nc.sync.dma_start(out=i15, in_=i15_d[:, :, :])
        ident128 = consts.tile([128, 128], F32R, tag="ident128")
        nc.sync.dma_start(out=ident128, in_=ident_d[:, :])
        d3c = consts.tile([128, 2, 256], F32R, tag="d3c")
        nc.sync.dma_start(out=d3c, in_=d3_d[:, :, :])

        # ---- persistent stat rows (per stats group) ----
        rows = [stats.tile([128, GSZ, 2], F32, tag="rows", name=f"rows{k}")
                for k in range(NGRP)]
        rsums = [stats.tile([1, GSZ, 2], F32, tag="rsums", name=f"rsums{k}")
                 for k in range(NGRP)]
        srow = [stats.tile([1, GSZ, 2], F32, tag="srow", name=f"srow{k}")
                for k in range(NGRP)]
        sc_all = [stats.tile([128, GSZ, 2], F32, tag="sc_all", name=f"sc{k}")
                  for k in range(NGRP)]
        out_all = stats.tile([128, GPC * 2], F32, tag="out_all")

        def load_and_accum(g):
            grp, j = divmod(g, GSZ)
            T = tpool.tile([128, 2, 256], BF16, tag="T", name="T")
            nc.sync.dma_start(out=T[:, 0, :], in_=fv[g])
            nc.sync.dma_start(out=T[:, 1, :], in_=nv[g])
            Ftile, Nztile = T[:, 0, :], T[:, 1, :]
            cs = ps8.tile([1, 512], F32, tag="ps", name="cs")
            nc.tensor.matmul(cs, ones128b, T[:, :, :],
                             start=True, stop=True)
            nc.vector.tensor_reduce(
                out=rsums[grp][0:1, j, :],
                in_=cs.rearrange("a (b c) -> a b c", c=256),
                axis=AX.X, op=OP.add)
            scr = scratch.tile([128, 256], F32, tag="sqscrA", name="scrA")
            nc.scalar.activation(out=scr, in_=Ftile, func=AF.Square,
                                 accum_out=rows[grp][:, j, 0:1])
            scr = scratch.tile([128, 256], F32, tag="sqscrA", name="scrA")
            nc.scalar.activation(out=scr, in_=Nztile, func=AF.Square,
                                 accum_out=rows[grp][:, j, 1:2])
            return T

        def stats_group(grp):
            # partition-sum all rows for the group, then batched scalar math
            tot_ps = ps8.tile([1, GSZ * 2], F32, tag="ps", name="tot_ps")
            nc.tensor.matmul(tot_ps, ones128f, rows[grp][:, :, :],
                             start=True, stop=True)
            rview = rsums[grp].rearrange("a g t -> a (g t)")
            sums_sq = small.tile([1, 2 * GSZ], F32, tag="ssq", name="ssq")
            nc.vector.tensor_mul(sums_sq, rview, rview)
            varnum = small.tile([1, 2 * GSZ], F32, tag="vn", name="vn")
            nc.vector.scalar_tensor_tensor(
                out=varnum, in0=sums_sq, scalar=-1.0 / NTOT,
                in1=tot_ps, op0=OP.mult, op1=OP.add)
            vview = varnum.rearrange("a (g t) -> a g t", t=2)
            rnv = small.tile([1, GSZ, 1], F32, tag="rnv", name="rnv")
            nc.vector.reciprocal(rnv, vview[:, :, 1:2])
            ratio = small.tile([1, GSZ, 1], F32, tag="ratio", name="ratio")
            nc.vector.tensor_mul(ratio, vview[:, :, 0:1], rnv)
            nc.scalar.activation(
                out=srow[grp][:, :, 0:1], in_=ratio, func=AF.Sqrt,
                scale=SNR_FACTOR)
            t3 = small.tile([1, GSZ, 1], F32, tag="t3", name="t3")
            nc.vector.tensor_mul(t3, srow[grp][:, :, 0:1], rsums[grp][:, :, 1:2])
            nc.scalar.mul(out=srow[grp][:, :, 1:2], in_=t3, mul=1.0 / NTOT)
            sc_ps = ps8.tile([128, GSZ * 2], F32, tag="ps", name="sc_ps")
            nc.tensor.matmul(sc_ps, ones1f, srow[grp][0:1, :, :],
                             start=True, stop=True)
            nc.scalar.copy(out=sc_all[grp],
                           in_=sc_ps.rearrange("p (g t) -> p g t", t=2))

        # =============== Phase A: load + stats (grouped) ===============
        Ts = []
        for grp in range(NGRP):
            for j in range(GSZ):
                Ts.append(load_and_accum(grp * GSZ + j))
            stats_group(grp)

        # =============== Phase B: two passes ===============
        # Pass 1 emits every graph's pre-NS work (independent across graphs),
        # pass 2 emits the 16 independent NS chains; this gives the static
        # scheduler a deep pool of ready work on every engine.
        def t_mms(Tp, YZt):
            # T = Z@Y into one [128,512] psum tile (halves m=0/m=1).
            # m=1 half: accumulate -3I on the PE so its drain is a pure
            # -0.5 scale on either engine; m=0 drains as STT on DVE.
            for kc in range(2):
                nc.tensor.matmul(
                    Tp[:, 0:256], YZt[kc][:, 256:384],
                    YZt[kc][:, 0:256],
                    start=(kc == 0), stop=(kc == 1))
            for kc in range(2):
                nc.tensor.matmul(
                    Tp[:, 256:512], YZt[kc][:, 384:512],
                    YZt[kc][:, 0:256],
                    start=(kc == 0), stop=False)
            nc.tensor.matmul(Tp[:, 256:512], ident128, d3c[:, 1, :],
                             start=False, stop=True)

        udrain_ctr = [0]

        def u_drain(U, Tp):
            # U = 1.5I - 0.5*T ; m=0 via DVE STT, m=1 (PE pre-subtracted 3I)
            # is a pure scale -- alternate it between Act and DVE.
            nc.vector.scalar_tensor_tensor(
                out=U[0], in0=Tp[:, 0:256], scalar=-0.5,
                in1=i15[:, 0, :], op0=OP.mult, op1=OP.add)
            udrain_ctr[0] += 1
            if udrain_ctr[0] % 2:
                nc.scalar.mul(out=U[1], in_=Tp[:, 256:512], mul=-0.5)
            else:
                nc.vector.tensor_scalar_mul(out=U[1], in0=Tp[:, 256:512],
                                            scalar1=-0.5)

        def w_mms(dsts, U, YZt):
            # W = U@[Y|Z] : lhsT = U chunks, rhs = full [128,512]
            for m in range(2):
                for kc in range(2):
                    nc.tensor.matmul(
                        dsts[m], U[kc][:, TS(m, 128)], YZt[kc],
                        start=(kc == 0), stop=(kc == 1))

        copy_ctr = [0]

        def cp_alt(dst, src):
            copy_ctr[0] += 1
            if copy_ctr[0] % 2:
                nc.scalar.copy(out=dst, in_=src)
            else:
                nc.vector.tensor_copy(out=dst, in_=src)

        # ---- pass 1, stage-major over blocks of 4 graphs: each engine's
        # queue gets 4 independent ops per stage, hiding dep latency ----
        state = [None] * GPC
        p1 = {}
        BLK = 4

        def p1_stageP(g):
            grp, j = divmod(g, GSZ)
            T = Ts[g]
            P = work.tile([128, 256], F32R, tag="P", name="P", bufs=8)
            nc.vector.scalar_tensor_tensor(
                out=P, in0=T[:, 1, :], scalar=sc_all[grp][:, j, 0:1],
                in1=T[:, 0, :], op0=OP.mult, op1=OP.add)
            p1[g] = {"P": P}

        def p1_bcast(g):
            bcast = ps8.tile([128, 256], F32, tag="ps", name="bcast")
            nc.tensor.matmul(bcast, oon_sq, p1[g]["P"], start=True, stop=True)
            p1[g]["bcast"] = bcast

        def p1_diff(g):
            P = p1[g]["P"]
            diff = work.tile([128, 256], F32R, tag="diff", name="diff", bufs=8)
            nc.vector.tensor_sub(diff, P, p1[g]["bcast"])
            p1[g]["diff"] = diff

        def p1_meansq(g):
            # mean (cols 0:2) and trace partial (col 2) pack into one psum
            P = p1[g]["P"]
            mtp = ps8.tile([128, 4], F32, tag="ps", name="mtp")
            for m in range(2):
                nc.tensor.matmul(
                    mtp[:, m : m + 1], P.bitcast(F32)[:, TS(m, 128)],
                    oon128f, start=True, stop=True)
            p1[g]["mtp"] = mtp
            scr = scratch.tile([128, 256], F32, tag="sqscrA", name="scrA")
            trrows = small.tile([128, 1], F32, tag="trrows", name="trrows")
            nc.scalar.activation(out=scr, in_=p1[g]["diff"], func=AF.Square,
                                 accum_out=trrows)
            p1[g]["trrows"] = trrows

        def p1_trace(g):
            mtp = p1[g]["mtp"]
            nc.tensor.matmul(mtp[:, 2:3], onesSQ, p1[g]["trrows"],
                             start=True, stop=True)
            rtr128 = small.tile([128, 1], F32, tag="rtr", name="rtr")
            nc.vector.reciprocal(rtr128, mtp[:, 2:3])
            sqtr128 = small.tile([128, 1], F32, tag="sqtr", name="sqtr")
            nc.scalar.activation(
                out=sqtr128, in_=mtp[:, 2:3], func=AF.Sqrt,
                scale=1.0 / (NNODE - 1))
            p1[g]["rtr"] = rtr128
            p1[g]["sqtr"] = sqtr128

        def p1_mv_cov(g):
            grp, j = divmod(g, GSZ)
            mv = small.tile([128, 2], BF16, tag="mv", name="mv")
            nc.vector.tensor_scalar(
                out=mv, in0=p1[g]["mtp"][:, 0:2],
                scalar1=sc_all[grp][:, j, 1:2],
                scalar2=p1[g]["sqtr"], op0=OP.subtract, op1=OP.mult)
            p1[g]["mv"] = mv
            diff = p1[g]["diff"]
            covp = ps8.tile([128, 512], F32, tag="ps", name="covp")
            for m in range(2):
                nc.tensor.matmul(covp[:, TS(m, 256)], diff[:, TS(m, 128)],
                                 diff, start=True, stop=True)
            p1[g]["covp"] = covp

        def p1_a_u0(g):
            Afull = nsp.tile([128, 512], BF16, tag="nsa", name="Afull", bufs=8)
            nc.scalar.mul(out=Afull, in_=p1[g]["covp"], mul=p1[g]["rtr"])
            A = [Afull[:, TS(m, 256)] for m in range(2)]
            YZ = [nsp.tile([128, 512], BF16, tag="ns", name="YZ", bufs=40)
                  for _ in range(2)]
            for m in range(2):
                nc.vector.scalar_tensor_tensor(
                    out=YZ[m][:, 256:512], in0=A[m], scalar=-0.5,
                    in1=i15[:, m, :], op0=OP.mult, op1=OP.add)
            p1[g]["A"] = A
            p1[g]["YZ"] = YZ

        def p1_y1(g):
            A, YZ = p1[g]["A"], p1[g]["YZ"]
            Yp = ps8.tile([128, 512], F32, tag="ps", name="Yp")
            for m in range(2):
                for kc in range(2):
                    nc.tensor.matmul(
                        Yp[:, TS(m, 256)],
                        YZ[kc][:, 256 + 128 * m : 384 + 128 * m], A[kc],
                        start=(kc == 0), stop=(kc == 1))
            cp_alt(YZ[0][:, 0:256], Yp[:, 0:256])
            cp_alt(YZ[1][:, 0:256], Yp[:, 256:512])
            state[g] = (YZ, p1[g]["mv"])

        P1_STAGES = [p1_stageP, p1_bcast, p1_diff, p1_meansq, p1_trace,
                     p1_mv_cov, p1_a_u0, p1_y1]

        YZs = [None] * GPC
        U3s = [None] * GPC

        def unit_pass1(b):
            for stage in P1_STAGES:
                for g in range(b * BLK, (b + 1) * BLK):
                    stage(g)
            for g in range(b * BLK, (b + 1) * BLK):
                YZs[g] = state[g][0]

        # full coupled iteration: T = Z@Y ; U = 1.5I-0.5T ; [Y|Z] = U@[Y|Z]
        def unit_iter(b):
            gs = range(b * BLK, (b + 1) * BLK)
            Tps, Us = {}, {}
            for g in gs:
                Tp = ps8.tile([128, 512], F32, tag="ps", name="Tp")
                t_mms(Tp, YZs[g])
                Tps[g] = Tp
            for g in gs:
                U = [nsp.tile([128, 256], BF16, tag="nsu", name="U", bufs=40)
                     for _ in range(2)]
                u_drain(U, Tps[g])
                Us[g] = U
            Wps = {}
            for g in gs:
                Wp = [ps8.tile([128, 512], F32, tag="ps", name="Wp")
                      for _ in range(2)]
                w_mms(Wp, Us[g], YZs[g])
                Wps[g] = Wp
            for g in gs:
                YZn = [nsp.tile([128, 512], BF16, tag="ns", name="YZ",
                                bufs=40) for _ in range(2)]
                cp_alt(YZn[0], Wps[g][0])
                cp_alt(YZn[1], Wps[g][1])
                YZs[g] = YZn

        # final stage (v2-style matrix form, stage-major within block):
        # T = Z4@Y4 ; U = 1.5I-0.5T ; YZY = U@Y4 ; out = YZY@mv
        def unit_final(b):
            gs = range(b * BLK, (b + 1) * BLK)
            Tps, Us, Yps, YZYs = {}, {}, {}, {}
            for g in gs:
                Tp = ps8.tile([128, 512], F32, tag="ps", name="Tp")
                t_mms(Tp, YZs[g])
                Tps[g] = Tp
            for g in gs:
                U = [nsp.tile([128, 256], BF16, tag="nsu", name="U", bufs=40)
                     for _ in range(2)]
                u_drain(U, Tps[g])
                Us[g] = U
            for g in gs:
                Yp = ps8.tile([128, 512], F32, tag="ps", name="Yp")
                for m in range(2):
                    for kc in range(2):
                        nc.tensor.matmul(
                            Yp[:, TS(m, 256)], Us[g][kc][:, TS(m, 128)],
                            YZs[g][kc][:, 0:256],
                            start=(kc == 0), stop=(kc == 1))
                Yps[g] = Yp
            for g in gs:
                YZY = [nsp.tile([128, 256], BF16, tag="nsu", name="YZY",
                                bufs=40) for _ in range(2)]
                cp_alt(YZY[0], Yps[g][:, 0:256])
                cp_alt(YZY[1], Yps[g][:, 256:512])
                YZYs[g] = YZY
            for g in gs:
                mv = state[g][1]
                outp = ps8.tile([128, 2], F32, tag="ps", name="outp")
                for m in range(2):
                    for kc in range(2):
                        nc.tensor.matmul(
                            outp[:, m : m + 1],
                            YZYs[g][kc][:, TS(m, 128)],
                            mv[:, kc : kc + 1],
                            start=(kc == 0), stop=(kc == 1))
                cp_alt(out_all[:, 2 * g : 2 * g + 2], outp)

        # ---- software-pipelined wavefront: interleave the elementwise-
        # heavy pass1 with the PE-heavy NS iterations across blocks ----
        NB = GPC // BLK
        UNITS = [unit_pass1, unit_iter, unit_iter, unit_iter, unit_final]
        K = len(UNITS)
        for t in range(NB + K - 1):
            for k in range(K - 1, -1, -1):
                b = t - k
                if 0 <= b < NB:
                    UNITS[k](b)

        # contiguous output DMA; host reorders to [GPC, D]
        nc.sync.dma_start(out=out_d[:, :], in_=out_all)


_NC_CACHE = None


def kernel(**inputs):
    global _NC_CACHE, LAST_RESULTS
    from ml_dtypes import bfloat16

    feat = np.ascontiguousarray(inputs["feat"]).astype(bfloat16)
    noise = np.ascontiguousarray(inputs["noise"]).astype(bfloat16)
    assert feat.shape == (B * NNODE, D) and noise.shape == (B * NNODE, D)

    if _NC_CACHE is None:
        _NC_CACHE = _build_bass()
    nc = _NC_CACHE

    rows = GPC * NNODE
    in_maps = [
        {
            "feat": feat[c * rows : (c + 1) * rows],
            "noise": noise[c * rows : (c + 1) * rows],
        }
        for c in range(N_CORES)
    ]
    res = run_bass_kernel_spmd(
        nc,
        in_maps,
        core_ids=list(range(N_CORES)),
        trace=bool(int(os.environ.get("DKE_TRACE", "0"))),
    )
    LAST_RESULTS = res
    # buf[p, 2g+m] -> out[g, m*128+p]
    out = np.concatenate(
        [
            m["out"].reshape(128, GPC, 2).transpose(1, 2, 0).reshape(GPC, D)
            for m in res.results
        ],
        axis=0,
    )
    return out.astype(np.float32)


if __name__ == "__main__":
    rng = np.random.default_rng(0)
    ins = {
        "batch_list": np.full((B,), NNODE, np.int32),
        "feat": rng.standard_normal((B * NNODE, D)).astype(np.float32),
        "noise": rng.standard_normal((B * NNODE, D)).astype(np.float32),
    }
    o = kernel(**ins)
    print(o.shape, o.dtype, np.abs(o).max())

